# revision 3
# baseline (speedup 1.0000x reference)
"""Trainium2 Bass kernel for nn_F2FBlock (2-layer SAGEConv GNN block).

Full inputs in, full output out. Internally: nodes sharded 6250/core across
8 NeuronCores (padded to 6272 = 49*128), edges sharded by dst ownership and
sorted by dst into 49 dst-tiles x CPT chunks of 128 edges. Per chunk the
kernel indirect-DMA-gathers 128 pre-transformed (h @ w_l.T, bf16) rows and
segment-sums them on the TensorEngine via a one-hot matmul. The two conv
layers exchange node features with an on-device AllGather.

reference math:
    shortcut = x @ sc_w.T + sc_b
    h = gelu(x @ dp_w.T + dp_b)
    h = mean_agg(h)@g1_lw.T + g1_lb + h@g1_rw.T          (SAGEConv 1)
    h = gelu(LN(h, n1_g, n1_b))
    h = mean_agg(h)@g2_lw.T + g2_lb + h@g2_rw.T          (SAGEConv 2)
    h = LN(h, n2_g, n2_b)
    out = gelu(h + shortcut)
where mean_agg(h)[i] = mean over {h[src] : (src,dst=i) in edges}.
Linearity lets us aggregate hl = h @ w_l.T and scale by 1/deg after.
"""

import numpy as np
import ml_dtypes

import concourse.bass as bass
import concourse.bacc as bacc
import concourse.tile as tile
import concourse.mybir as mybir
from concourse.masks import make_identity

P = 128
D = 128
N = 50000
NCORE = 8
OWN = N // NCORE            # 6250 owned nodes per core
NT = (OWN + P - 1) // P     # 49 dst tiles per core
SLAB = NT * P               # 6272 padded rows per core
NPAD = SLAB * NCORE         # 50176 rows in gathered tables
EPS = 1e-5

F32 = mybir.dt.float32
BF16 = mybir.dt.bfloat16
I32 = mybir.dt.int32
AF = mybir.ActivationFunctionType
ALU = mybir.AluOpType


def _build_nc(cpt: int, reps: int = 1):
    """Build the Bass module. cpt = chunks (of 128 edges) per dst tile.
    reps > 1 repeats the whole computation (for timing differentials)."""
    nchunk = NT * cpt
    nc = bacc.Bacc("TRN2", target_bir_lowering=False, debug=False,
                   num_devices=NCORE)

    # ---- I/O ----
    x_t = nc.dram_tensor("x_t", [P, SLAB], F32, kind="ExternalInput")
    src_idx = nc.dram_tensor("src_idx", [P, nchunk], I32, kind="ExternalInput")
    dst_loc = nc.dram_tensor("dst_loc", [P, nchunk], F32, kind="ExternalInput")
    inv_cnt = nc.dram_tensor("inv_cnt", [P, NT], F32, kind="ExternalInput")
    iota_in = nc.dram_tensor("iota_in", [P, P], F32, kind="ExternalInput")
    # weight matrices, already transposed to [fin, fout] on host
    w_names = ["w_dp", "w_sc", "w_g1l", "w_g1r", "w_g2l", "w_g2r"]
    w_in = {n: nc.dram_tensor(n, [D, D], F32, kind="ExternalInput") for n in w_names}
    dp_b = nc.dram_tensor("dp_b", [D, 1], F32, kind="ExternalInput")
    # feature-axis vectors replicated to [P, D] on host
    r_names = ["sc_b", "g1_lb", "g2_lb", "n1_g", "n1_b", "n2_g", "n2_b"]
    r_in = {n: nc.dram_tensor(n, [P, D], F32, kind="ExternalInput") for n in r_names}
    out = nc.dram_tensor("out", [SLAB, D], F32, kind="ExternalOutput")

    with tile.TileContext(nc) as tc:
        with (
            tc.tile_pool(name="const", bufs=1) as cp,
            tc.tile_pool(name="work", bufs=4) as wp,
            tc.tile_pool(name="msgs", bufs=12) as mp,
            tc.tile_pool(name="oneh", bufs=12) as op_,
            tc.tile_pool(name="small", bufs=4) as sp,
            tc.tile_pool(name="psA", bufs=2, space="PSUM") as pA,
            tc.tile_pool(name="psB", bufs=4, space="PSUM") as pB,
            tc.tile_pool(name="dram", bufs=1, space="DRAM") as dp_,
        ):
            # ---- constants into SBUF ----
            xt_s = cp.tile([P, SLAB], F32, tag="xt")
            nc.sync.dma_start(out=xt_s[:], in_=x_t[:])
            si_s = cp.tile([P, nchunk], I32, tag="si")
            nc.sync.dma_start(out=si_s[:], in_=src_idx[:])
            dl_s = cp.tile([P, nchunk], F32, tag="dl")
            nc.sync.dma_start(out=dl_s[:], in_=dst_loc[:])
            ic_s = cp.tile([P, NT], F32, tag="ic")
            nc.sync.dma_start(out=ic_s[:], in_=inv_cnt[:])
            io_s = cp.tile([P, P], F32, tag="io")
            nc.sync.dma_start(out=io_s[:], in_=iota_in[:])
            w_s = {}
            for n in w_names:
                w_s[n] = cp.tile([D, D], F32, tag=n, name=n)
                nc.sync.dma_start(out=w_s[n][:], in_=w_in[n][:])
            dpb_s = cp.tile([D, 1], F32, tag="dpb")
            nc.sync.dma_start(out=dpb_s[:], in_=dp_b[:])
            r_s = {}
            for n in r_names:
                r_s[n] = cp.tile([P, D], F32, tag=n, name=n)
                nc.sync.dma_start(out=r_s[n][:], in_=r_in[n][:])
            ident = cp.tile([P, P], F32, tag="ident")
            make_identity(nc, ident[:])

            # internal DRAM state
            hl1slab = dp_.tile([SLAB, D], BF16)
            hl1full = dp_.tile([NPAD, D], BF16)
            hl2slab = dp_.tile([SLAB, D], BF16)
            hl2full = dp_.tile([NPAD, D], BF16)
            h0r_d = dp_.tile([SLAB, D], F32)
            shct_d = dp_.tile([SLAB, D], F32)

            def layer_norm(h, gamma_t, beta_t):
                """LN over free dim of node-major h [P, D]."""
                scratch = wp.tile([P, D], F32, tag="lnscr")
                sumsq = sp.tile([P, 1], F32, tag="sumsq")
                nc.scalar.activation(out=scratch[:], in_=h[:], func=AF.Square,
                                     accum_out=sumsq[:])
                ssum = sp.tile([P, 1], F32, tag="ssum")
                nc.vector.tensor_reduce(out=ssum[:], in_=h[:],
                                        axis=mybir.AxisListType.X, op=ALU.add)
                mu = sp.tile([P, 1], F32, tag="mu")
                nc.vector.tensor_scalar_mul(out=mu[:], in0=ssum[:], scalar1=1.0 / D)
                musq = sp.tile([P, 1], F32, tag="musq")
                nc.vector.tensor_tensor(out=musq[:], in0=mu[:], in1=mu[:], op=ALU.mult)
                var = sp.tile([P, 1], F32, tag="var")
                nc.vector.scalar_tensor_tensor(out=var[:], in0=sumsq[:],
                                               scalar=1.0 / D, in1=musq[:],
                                               op0=ALU.mult, op1=ALU.subtract)
                nc.vector.tensor_scalar_add(out=var[:], in0=var[:], scalar1=EPS)
                sd = sp.tile([P, 1], F32, tag="sd")
                nc.scalar.activation(out=sd[:], in_=var[:], func=AF.Sqrt)
                rstd = sp.tile([P, 1], F32, tag="rstd")
                nc.vector.reciprocal(out=rstd[:], in_=sd[:])
                nmr = sp.tile([P, 1], F32, tag="nmr")
                nc.vector.scalar_tensor_tensor(out=nmr[:], in0=mu[:], scalar=-1.0,
                                               in1=rstd[:], op0=ALU.mult, op1=ALU.mult)
                hn = wp.tile([P, D], F32, tag="hn")
                nc.scalar.activation(out=hn[:], in_=h[:], func=AF.Identity,
                                     scale=rstd[:], bias=nmr[:])
                hg = wp.tile([P, D], F32, tag="hg")
                nc.vector.tensor_tensor(out=hg[:], in0=hn[:], in1=gamma_t[:], op=ALU.mult)
                nc.vector.tensor_tensor(out=hg[:], in0=hg[:], in1=beta_t[:], op=ALU.add)
                return hg

            def aggregate(table, t):
                """Segment-sum chunks of dst tile t from bf16 table; returns
                mean-scaled f32 sbuf tile [P, D]."""
                ps = pA.tile([P, D], F32, space="PSUM", tag="agg")
                for k in range(cpt):
                    c = t * cpt + k
                    msgs = mp.tile([P, D], BF16, tag="msgs")
                    nc.gpsimd.indirect_dma_start(
                        out=msgs[:], out_offset=None, in_=table[:],
                        in_offset=bass.IndirectOffsetOnAxis(
                            ap=si_s[:, c:c + 1], axis=0))
                    w1 = op_.tile([P, P], BF16, tag="oneh")
                    nc.vector.tensor_tensor(
                        out=w1[:], in0=dl_s[:, c:c + 1].to_broadcast([P, P]),
                        in1=io_s[:], op=ALU.is_equal)
                    nc.tensor.matmul(ps[:], lhsT=w1[:], rhs=msgs[:],
                                     start=(k == 0), stop=(k == cpt - 1))
                h = wp.tile([P, D], F32, tag="hagg")
                nc.scalar.mul(out=h[:], in_=ps[:], mul=ic_s[:, t:t + 1])
                return h

            for _rep in range(reps):
                # ---- phase B1: hl1 chain only (feeds AllGather 1 asap) ----
                h0fm_tiles = []
                for i in range(NT):
                    xt_i = xt_s[:, i * P:(i + 1) * P]
                    ph = pB.tile([P, P], F32, space="PSUM", tag="pd")
                    nc.tensor.matmul(ph[:], lhsT=w_s["w_dp"][:], rhs=xt_i,
                                     start=True, stop=True)
                    h0fm = cp.tile([P, P], F32, tag=f"h0fm{i}", name=f"h0fm{i}")
                    nc.scalar.activation(out=h0fm[:], in_=ph[:], func=AF.Gelu,
                                         bias=dpb_s[:])
                    h0fm_tiles.append(h0fm)
                    p2 = pB.tile([P, P], F32, space="PSUM", tag="pd")
                    nc.tensor.matmul(p2[:], lhsT=h0fm[:], rhs=w_s["w_g1l"][:],
                                     start=True, stop=True)
                    hl1bf = wp.tile([P, P], BF16, tag="hl1bf")
                    nc.vector.tensor_copy(out=hl1bf[:], in_=p2[:])
                    nc.sync.dma_start(out=hl1slab[i * P:(i + 1) * P, :], in_=hl1bf[:])

                # ---- AllGather 1 ----
                nc.gpsimd.collective_compute(
                    "AllGather", ALU.bypass,
                    replica_groups=[list(range(NCORE))],
                    ins=[hl1slab.opt()], outs=[hl1full.opt()])

                # ---- phase B2: shortcut + r-path, overlaps AllGather 1 ----
                for i in range(NT):
                    xt_i = xt_s[:, i * P:(i + 1) * P]
                    h0fm = h0fm_tiles[i]
                    p3 = pB.tile([P, P], F32, space="PSUM", tag="pd")
                    nc.tensor.matmul(p3[:], lhsT=h0fm[:], rhs=w_s["w_g1r"][:],
                                     start=True, stop=True)
                    h0r_s = wp.tile([P, P], F32, tag="h0rs")
                    nc.vector.tensor_tensor(out=h0r_s[:], in0=p3[:],
                                            in1=r_s["g1_lb"][:], op=ALU.add)
                    nc.sync.dma_start(out=h0r_d[i * P:(i + 1) * P, :], in_=h0r_s[:])
                    p4 = pB.tile([P, P], F32, space="PSUM", tag="pd")
                    nc.tensor.matmul(p4[:], lhsT=xt_i, rhs=w_s["w_sc"][:],
                                     start=True, stop=True)
                    sc_s = wp.tile([P, P], F32, tag="scs")
                    nc.vector.tensor_tensor(out=sc_s[:], in0=p4[:],
                                            in1=r_s["sc_b"][:], op=ALU.add)
                    nc.sync.dma_start(out=shct_d[i * P:(i + 1) * P, :], in_=sc_s[:])

                # ---- layer 1 aggregation + assembly ----
                h1fm_tiles = []
                for t in range(NT):
                    h1 = aggregate(hl1full, t)
                    h0r_t = wp.tile([P, D], F32, tag="h0rt")
                    nc.sync.dma_start(out=h0r_t[:], in_=h0r_d[t * P:(t + 1) * P, :])
                    nc.vector.tensor_tensor(out=h1[:], in0=h1[:], in1=h0r_t[:], op=ALU.add)
                    h1ln = layer_norm(h1, r_s["n1_g"], r_s["n1_b"])
                    h1g = wp.tile([P, D], F32, tag="hgel")
                    nc.scalar.activation(out=h1g[:], in_=h1ln[:], func=AF.Gelu)
                    tp = pB.tile([P, P], F32, space="PSUM", tag="pd")
                    nc.tensor.transpose(out=tp[:], in_=h1g[:], identity=ident[:])
                    h1fm = cp.tile([P, P], F32, tag=f"h1fm{t}", name=f"h1fm{t}")
                    nc.vector.tensor_copy(out=h1fm[:], in_=tp[:])
                    h1fm_tiles.append(h1fm)
                    p5 = pB.tile([P, P], F32, space="PSUM", tag="pd")
                    nc.tensor.matmul(p5[:], lhsT=h1fm[:], rhs=w_s["w_g2l"][:],
                                     start=True, stop=True)
                    hl2bf = wp.tile([P, P], BF16, tag="hl2bf")
                    nc.vector.tensor_copy(out=hl2bf[:], in_=p5[:])
                    nc.sync.dma_start(out=hl2slab[t * P:(t + 1) * P, :], in_=hl2bf[:])

                # ---- AllGather 2 ----
                nc.gpsimd.collective_compute(
                    "AllGather", ALU.bypass,
                    replica_groups=[list(range(NCORE))],
                    ins=[hl2slab.opt()], outs=[hl2full.opt()])

                # ---- layer 2 aggregation + assembly + output ----
                for t in range(NT):
                    h2 = aggregate(hl2full, t)
                    p6 = pB.tile([P, P], F32, space="PSUM", tag="pd")
                    nc.tensor.matmul(p6[:], lhsT=h1fm_tiles[t][:], rhs=w_s["w_g2r"][:],
                                     start=True, stop=True)
                    h1r_s = wp.tile([P, P], F32, tag="h1rs")
                    nc.vector.tensor_tensor(out=h1r_s[:], in0=p6[:],
                                            in1=r_s["g2_lb"][:], op=ALU.add)
                    nc.vector.tensor_tensor(out=h2[:], in0=h2[:], in1=h1r_s[:], op=ALU.add)
                    h2n = layer_norm(h2, r_s["n2_g"], r_s["n2_b"])
                    sh_t = wp.tile([P, D], F32, tag="sht")
                    nc.sync.dma_start(out=sh_t[:], in_=shct_d[t * P:(t + 1) * P, :])
                    nc.vector.tensor_tensor(out=h2n[:], in0=h2n[:], in1=sh_t[:], op=ALU.add)
                    o_t = wp.tile([P, D], F32, tag="ot")
                    nc.scalar.activation(out=o_t[:], in_=h2n[:], func=AF.Gelu)
                    nc.sync.dma_start(out=out[t * P:(t + 1) * P, :], in_=o_t[:])

    nc.compile()
    return nc


# ---------------------------------------------------------------------------
# host side: preprocessing + PJRT runner
# ---------------------------------------------------------------------------

class _Runner:
    """Reusable jitted PJRT executor for a compiled Bass module (axon)."""

    def __init__(self, nc, n_cores):
        import jax
        from jax.sharding import Mesh, PartitionSpec
        from jax.experimental.shard_map import shard_map
        from concourse.bass2jax import (_bass_exec_p, install_neuronx_cc_hook,
                                        partition_id_tensor)
        self.jax = jax
        install_neuronx_cc_hook()
        self.n_cores = n_cores
        pname = nc.partition_id_tensor.name if nc.partition_id_tensor else None
        in_names, out_names, out_avals, zero_outs = [], [], [], []
        for alloc in nc.m.functions[0].allocations:
            if not isinstance(alloc, mybir.MemoryLocationSet):
                continue
            name = alloc.memorylocations[0].name
            if alloc.kind == "ExternalInput":
                if name != pname:
                    in_names.append(name)
            elif alloc.kind == "ExternalOutput":
                shape = tuple(alloc.tensor_shape)
                dtype = mybir.dt.np(alloc.dtype)
                out_names.append(name)
                out_avals.append(jax.core.ShapedArray(shape, dtype))
                zero_outs.append(np.zeros(shape, dtype))
        self.in_names, self.out_names = in_names, out_names
        self.out_avals, self.zero_outs = out_avals, zero_outs
        n_params, n_outs = len(in_names), len(out_names)
        all_in = list(in_names) + list(out_names)
        if pname is not None:
            all_in.append(pname)

        def _body(*args):
            operands = list(args)
            if pname is not None:
                operands.append(partition_id_tensor())
            outs = _bass_exec_p.bind(
                *operands, out_avals=tuple(out_avals), in_names=tuple(all_in),
                out_names=tuple(out_names), lowering_input_output_aliases=(),
                sim_require_finite=False, sim_require_nnan=False, nc=nc)
            return tuple(outs)

        devices = jax.devices()[:n_cores]
        mesh = Mesh(np.asarray(devices), ("core",))
        self.mesh = mesh
        in_specs = (PartitionSpec("core"),) * (n_params + n_outs)
        out_specs = (PartitionSpec("core"),) * n_outs
        self.fn = jax.jit(
            shard_map(_body, mesh=mesh, in_specs=in_specs,
                      out_specs=out_specs, check_rep=False),
            keep_unused=True)

    def prep(self, in_maps):
        """Transfer inputs to device once; returns device-resident args."""
        n = self.n_cores
        args = [np.concatenate([np.asarray(in_maps[c][nm]) for c in range(n)], 0)
                for nm in self.in_names]
        args += [np.zeros((n * z.shape[0], *z.shape[1:]), z.dtype)
                 for z in self.zero_outs]
        jax = self.jax
        from jax.sharding import NamedSharding, PartitionSpec
        sh = NamedSharding(self.mesh, PartitionSpec("core"))
        dev_args = [jax.device_put(a, sh) for a in args]
        for a in dev_args:
            a.block_until_ready()
        return dev_args

    def run_dev(self, dev_args):
        """Run on pre-transferred args; blocks; leaves outputs on device."""
        outs = self.fn(*dev_args)
        for o in outs:
            o.block_until_ready()
        return outs

    def run(self, in_maps):
        n = self.n_cores
        out_arrs = self.run_dev(self.prep(in_maps))
        return [
            {nm: np.asarray(out_arrs[i]).reshape(n, *self.out_avals[i].shape)[c]
             for i, nm in enumerate(self.out_names)}
            for c in range(n)
        ]


_CACHE = {}


def _get_runner(cpt, reps=1):
    key = (cpt, reps)
    if key not in _CACHE:
        nc = _build_nc(cpt, reps)
        _CACHE[key] = _Runner(nc, NCORE)
    return _CACHE[key]


def _preprocess(x, edges, dp_w, dp_b, sc_w, sc_b, g1_lw, g1_lb, g1_rw, n1_g,
                n1_b, g2_lw, g2_lb, g2_rw, n2_g, n2_b):
    src = np.asarray(edges[0], dtype=np.int64)
    dst = np.asarray(edges[1], dtype=np.int64)
    x = np.asarray(x, dtype=np.float32)

    cnt = np.bincount(dst, minlength=N).astype(np.float32)
    inv = 1.0 / np.maximum(cnt, 1.0)
    # padded node id for the gathered tables
    pid = (src // OWN) * SLAB + (src % OWN)

    core_of = dst // OWN
    dloc_all = dst % OWN

    per_core = []
    cpt_needed = 0
    order_all = np.argsort(dloc_all + core_of * OWN, kind="stable")
    for c in range(NCORE):
        m = core_of == c
        idx = np.flatnonzero(m)
        o = idx[np.argsort(dloc_all[idx], kind="stable")]
        s_c, d_c = pid[o], dloc_all[o]
        # per-tile boundaries
        tile_id = d_c // P
        counts = np.bincount(tile_id, minlength=NT)
        cpt_needed = max(cpt_needed, int(np.ceil(counts.max() / P)))
        per_core.append((s_c, d_c, tile_id, counts))

    cpt = max(12, cpt_needed)
    nchunk = NT * cpt

    ins = []
    for c in range(NCORE):
        s_c, d_c, tile_id, counts = per_core[c]
        src_arr = np.zeros((P, nchunk), np.int32)
        dl_arr = np.full((P, nchunk), -1.0, np.float32)
        starts = np.concatenate([[0], np.cumsum(counts)])
        for t in range(NT):
            lo, hi = starts[t], starts[t + 1]
            ne = hi - lo
            if ne == 0:
                continue
            s_t = s_c[lo:hi]
            d_t = (d_c[lo:hi] - t * P).astype(np.float32)
            full, rem = divmod(ne, P)
            col0 = t * cpt
            if full:
                src_arr[:, col0:col0 + full] = s_t[:full * P].reshape(full, P).T
                dl_arr[:, col0:col0 + full] = d_t[:full * P].reshape(full, P).T
            if rem:
                src_arr[:rem, col0 + full] = s_t[full * P:]
                dl_arr[:rem, col0 + full] = d_t[full * P:]

        icnt = np.ones((P, NT), np.float32)
        base = c * OWN
        for t in range(NT):
            n0 = t * P
            n1 = min(n0 + P, OWN)
            icnt[:n1 - n0, t] = inv[base + n0:base + n1]

        x_slab = np.zeros((SLAB, D), np.float32)
        x_slab[:OWN] = x[base:base + OWN]

        ins.append({
            "x_t": np.ascontiguousarray(x_slab.T),
            "src_idx": src_arr,
            "dst_loc": dl_arr,
            "inv_cnt": icnt,
        })

    iota = np.tile(np.arange(P, dtype=np.float32), (P, 1))
    shared = {
        "iota_in": iota,
        "w_dp": np.ascontiguousarray(np.asarray(dp_w, np.float32).T),
        "w_sc": np.ascontiguousarray(np.asarray(sc_w, np.float32).T),
        "w_g1l": np.ascontiguousarray(np.asarray(g1_lw, np.float32).T),
        "w_g1r": np.ascontiguousarray(np.asarray(g1_rw, np.float32).T),
        "w_g2l": np.ascontiguousarray(np.asarray(g2_lw, np.float32).T),
        "w_g2r": np.ascontiguousarray(np.asarray(g2_rw, np.float32).T),
        "dp_b": np.asarray(dp_b, np.float32).reshape(D, 1),
        "sc_b": np.tile(np.asarray(sc_b, np.float32), (P, 1)),
        "g1_lb": np.tile(np.asarray(g1_lb, np.float32), (P, 1)),
        "g2_lb": np.tile(np.asarray(g2_lb, np.float32), (P, 1)),
        "n1_g": np.tile(np.asarray(n1_g, np.float32), (P, 1)),
        "n1_b": np.tile(np.asarray(n1_b, np.float32), (P, 1)),
        "n2_g": np.tile(np.asarray(n2_g, np.float32), (P, 1)),
        "n2_b": np.tile(np.asarray(n2_b, np.float32), (P, 1)),
    }
    for m in ins:
        m.update(shared)
    return ins, cpt


def kernel(**inputs) -> np.ndarray:
    in_maps, cpt = _preprocess(**inputs)
    runner = _get_runner(cpt)
    res = runner.run(in_maps)
    return np.concatenate([res[c]["out"][:OWN] for c in range(NCORE)], axis=0)

